# revision 79
# baseline (speedup 1.0000x reference)
"""Masked dot-product attention (B=64, Lq=Lk=1024, d=64, fp32) on 8 TRN2 cores.

v2 strategy (batch-parallel, 8 batch slots per core), all fp16 operands:
  - Host folds 1/sqrt(d) into Q, converts Q/K/V to fp16, and packs the key
    mask (0 valid / -1e6 masked) as contraction row 64, so masked scores come
    from one matmul: S^T[k, q] = sum_d K[k,d]*Q[q,d]/8 + mask[k]*1.
    fp16 matmuls run at 1 cycle/row for ANY output width (unlike fp32r's
    >=256 requirement), enabling narrow O-matmuls.
  - exp is split across TWO engines, alternating whole k-tiles:
      * ACT: exact exp (PSUM fp32 -> SBUF fp16).
      * DVE: Schraudolph fast-exp in one tensor_scalar:
          bits_i16 = round_sat(s * 1477.3197 + 15315.25)  (= fp16 bits of
          ~exp(s), max rel err 3%). Saturation maps masked -1e6 scores to
          -32768 = fp16 -0.0, i.e. exactly zero attention weight.
  - O accumulates per q-tile: out[128q, 65] += P^T[k, q-tile]^T @ [V|1]
    (65 cols: 64 outputs + softmax denominator). PSUM allows only ONE open
    accumulation group per bank, so the 8 q-groups live at 65-col strides in
    2 banks ([128, 2, 512] tile); each bank's group sequence is opened by a
    single start=True matmul whose rhs is [V|1|zeros] (writes group 0/4's
    data and zeroes the other groups' region), every other matmul
    accumulates with start=False, and the bank's last matmul sets stop=True.
  - DVE compacts [128, 2, 260] PSUM fp32 -> SBUF fp16 for a single
    contiguous output DMA; the divide + final transpose happen on the host.
  - Raggedness: batches sorted by active k-tile count and dealt across cores
    (slot s runs max-of-group tiles on every core); dead k-tiles skipped.
  - Emission is software-pipelined: S-matmul of tile i+1 precedes the
    O-matmuls of tile i so PE never waits on exp.
"""

import numpy as np

import concourse.bass as bass
import concourse.mybir as mybir
import concourse.tile as tile
from concourse import bacc
from concourse.bass_utils import run_bass_kernel_spmd

N_CORES = 8
B = 64
L = 1024
D = 64
BPC = B // N_CORES   # batch slots per core
KT = L // 128        # max k-tiles per batch
NEG_INF = -1000000.0

F16 = mybir.dt.float16
F32 = mybir.dt.float32
I16 = mybir.dt.int16

# Schraudolph fp16 fast-exp constants (C=44.75 minimax-tuned: 3.0% max err)
EXP_A = 1477.3197
EXP_B = 15315.25

QKM_W = 2 * L        # [ktile0 (128) | qt (1024) | ktiles 1..7 (896)]

_prog_cache = {}


def _build_program(ns):
    """ns: per-slot k-tile counts (tuple of BPC ints in 1..KT)."""
    nc = bacc.Bacc("TRN2", target_bir_lowering=False, debug=False,
                   num_devices=N_CORES)
    qkm_d = nc.dram_tensor("qkm", [BPC, D + 1, QKM_W], F16,
                           kind="ExternalInput")
    # vp packs [vt0 | zeros(195) | vt1 | vt2 | ...]: the kt==0 slice is the
    # 260-wide zero-padded rhs of the bank-opening matmuls.
    vp_d = nc.dram_tensor("vp", [BPC, 128, 260 + (KT - 1) * (D + 1)], F16,
                          kind="ExternalInput")
    # fp32: the device ships raw CUMULATIVE accumulator snapshots (batch s's
    # O-matmuls accumulate onto batch s-1's results instead of paying
    # bank-zeroing matmuls); the host recovers per-batch values by
    # differencing, which needs fp32 precision.
    o_d = nc.dram_tensor("o", [BPC, 128, 2, 4 * (D + 1)], F32,
                         kind="ExternalOutput")
    # the last batch zero-opens its banks (no stale base), so its output is
    # exact and ships fp16 -- halving the tail DMA that ends the kernel
    o16_d = nc.dram_tensor("o16", [128, 2, 4 * (D + 1)], F16,
                           kind="ExternalOutput")

    # Flat (batch, ktile) job list for cross-batch software pipelining.
    jobs = [(b, kt) for b in range(BPC) for kt in range(ns[b])]
    n_jobs = len(jobs)
    # Whole-tile exp alternating between ACT (exact) and DVE (Schraudolph):
    # one instruction per tile amortizes the fixed per-instruction engine
    # overhead (~185ns ACT / ~125ns DVE). DVE is ~1.15x slower per tile, so
    # it gets the smaller share, evenly spread. Output copies are split per
    # bank: bank A on ACT, bank B on DVE.
    # DVE is the saturated engine: front-load its tiles (odd ring positions
    # from g=1) so it starts as soon as the second S-matmul lands instead of
    # idling through the pipeline warmup.
    n_dve = round(n_jobs * 0.4865)
    dve_tile = [False] * n_jobs
    if n_dve > 0 and n_jobs > 1:
        for i in range(n_dve):
            dve_tile[1 + (i * (n_jobs - 1)) // n_dve] = True

    with tile.TileContext(nc) as tc:
        with (
            tc.tile_pool(name="qkm", bufs=3) as qkm_pool,
            tc.tile_pool(name="vpp", bufs=3) as vp_pool,
            tc.tile_pool(name="pt", bufs=7) as pt_pool,
            tc.tile_pool(name="osb", bufs=3) as osb_pool,
            tc.tile_pool(name="sp", bufs=3, space="PSUM") as sp_pool,
            tc.tile_pool(name="op", bufs=1, space="PSUM") as op_pool,
        ):
            qkm_s = [None] * BPC
            vp_s = [None] * BPC
            opsum = [None] * BPC
            pt_of_job = [None] * n_jobs
            started = [False] * (BPC + 1)

            # warm the ACT exp table (1.3us load) during the DMA-latency head
            sc = qkm_pool.tile([128, 1], F32, tag="sc", name="sc")
            nc.gpsimd.memset(sc[:], 0.0)
            nc.scalar.activation(sc[:], sc[:],
                                 mybir.ActivationFunctionType.Exp)

            def ktm_sl(b, kt):
                if kt == 0:
                    return qkm_s[b][:, :128]
                o = 128 + L + (kt - 1) * 128
                return qkm_s[b][:, o:o + 128]

            def start_batch(b):
                nkt = ns[b]
                end = 128 + L + (nkt - 1) * 128
                vend = 260 + (nkt - 1) * (D + 1)
                qkm = qkm_pool.tile([D + 1, QKM_W], F16, tag="qkm",
                                    name=f"qkm{b}")
                vp = vp_pool.tile([128, 260 + (KT - 1) * (D + 1)], F16,
                                  tag="vp", name=f"vp{b}")
                if b == 0:
                    # head: ktile0 + first half of qt -> first matmuls sooner
                    nc.sync.dma_start(qkm[:, :640], qkm_d[b][:, :640])
                    nc.sync.dma_start(qkm[:, 640:end], qkm_d[b][:, 640:end])
                else:
                    nc.sync.dma_start(qkm[:, :end], qkm_d[b][:, :end])
                nc.sync.dma_start(vp[:, :vend], vp_d[b][:, :vend])
                qkm_s[b] = qkm
                vp_s[b] = vp

            def ensure_started(b):
                if b < BPC and not started[b]:
                    started[b] = True
                    start_batch(b)

            def s_and_exp(g):
                b, kt = jobs[g]
                ensure_started(b)
                # prefetch inputs for every batch reached within 6 jobs
                tb = jobs[min(g + 6, n_jobs - 1)][0]
                for bb in range(b + 1, tb + 1):
                    ensure_started(bb)
                nkt = ns[b]
                qt = qkm_s[b][:, 128:128 + L]
                sp = sp_pool.tile([128, L], F32, tag="sp", name=f"sp{g}")
                pt = pt_pool.tile([128, L], F16, tag="pt", name=f"pt{g}")
                pt_of_job[g] = pt
                if g == 0:
                    # quarter-width matmuls to prime the pipeline
                    for qo in range(0, L, 256):
                        nc.tensor.matmul(sp[:, qo:qo + 256], ktm_sl(b, kt),
                                         qt[:, qo:qo + 256],
                                         start=True, stop=True)
                else:
                    for qo in range(0, L, 512):
                        nc.tensor.matmul(sp[:, qo:qo + 512], ktm_sl(b, kt),
                                         qt[:, qo:qo + 512],
                                         start=True, stop=True)
                if dve_tile[g] and g > 0:
                    nc.vector.tensor_scalar(
                        pt.bitcast(I16)[:], sp[:], EXP_A, EXP_B,
                        mybir.AluOpType.mult, mybir.AluOpType.add)
                elif g == 0:
                    # split exp so the first O-matmuls can start sooner
                    for eo in range(0, L, 512):
                        nc.scalar.activation(
                            pt[:, eo:eo + 512], sp[:, eo:eo + 512],
                            mybir.ActivationFunctionType.Exp)
                else:
                    nc.scalar.activation(pt[:], sp[:],
                                         mybir.ActivationFunctionType.Exp)

            def o_mm(g):
                b, kt = jobs[g]
                nkt = ns[b]
                if kt == 0:
                    opsum[b] = op_pool.tile([128, 2, 512], F32, tag="op",
                                            name=f"op{b}")
                pt = pt_of_job[g]
                last = kt == nkt - 1
                if kt == 0 and (b == 0 or b == BPC - 1):
                    # PSUM is undefined at program start (b==0), and the
                    # last batch opts out of the stale-accumulate chain so
                    # its output is exact fp16: open each bank with a
                    # zero-padded matmul (group 4h's data + zeroed region).
                    # All other batches accumulate onto the previous batch's
                    # (already copied-out) results; the host undoes the
                    # running sum by differencing.
                    vz = vp_s[b][:, :260]  # [V|1|zeros(195)]
                    for h in range(2):
                        nc.tensor.matmul(
                            opsum[b][:, h, :260],
                            pt[:, h * 512:h * 512 + 128], vz,
                            start=True, stop=False,
                            skip_group_check=True)
                    qlist = [1, 2, 3, 5, 6, 7]
                else:
                    qlist = list(range(8))
                vt = vp_s[b][:, kt * (D + 1) + 195:kt * (D + 1) + 260] \
                    if kt > 0 else vp_s[b][:, :D + 1]
                for q in qlist:
                    h, j = divmod(q, 4)
                    nc.tensor.matmul(
                        opsum[b][:, h, j * (D + 1):(j + 1) * (D + 1)],
                        pt[:, q * 128:(q + 1) * 128],
                        vt,
                        start=False,
                        stop=last and (j == 3),
                        skip_group_check=True,
                    )
                pt_of_job[g] = None
                if last and b == BPC - 1:
                    # tail: per-bank copies in parallel (ACT || DVE), fp16
                    osbL = osb_pool.tile([128, 2, 4 * (D + 1)], F16,
                                         tag="osbL", name="osbL")
                    nc.scalar.copy(osbL[:, 0, :], opsum[b][:, 0, :4 * (D + 1)])
                    nc.vector.tensor_copy(osbL[:, 1, :],
                                          opsum[b][:, 1, :4 * (D + 1)])
                    nc.sync.dma_start(o16_d[:], osbL[:])
                elif last:
                    # ONE copy instruction per batch, alternating engines:
                    # halves the per-instruction overhead vs per-bank copies
                    osb = osb_pool.tile([128, 2, 4 * (D + 1)], F32,
                                        tag="osb", name=f"osb{b}")
                    if b % 2 == 0:
                        nc.scalar.copy(osb[:], opsum[b][:, :, :4 * (D + 1)])
                        nc.gpsimd.dma_start(o_d[b], osb[:])
                    else:
                        nc.vector.tensor_copy(osb[:],
                                              opsum[b][:, :, :4 * (D + 1)])
                        nc.sync.dma_start(o_d[b], osb[:])

            # pipeline: O of job g trails the S/exp stream by LOOKAHEAD jobs
            # so the PE wait-queue never blocks on an in-flight exp. O (and
            # the trailing copies) are emitted BEFORE the S/exp at the same
            # position so copies land ahead of exp in the ACT/DVE queues.
            LOOKAHEAD = 5
            for g in range(n_jobs):
                if g >= LOOKAHEAD:
                    o_mm(g - LOOKAHEAD)
                s_and_exp(g)
            for g in range(max(0, n_jobs - LOOKAHEAD), n_jobs):
                o_mm(g)

    nc.compile()
    return nc


def get_program(ns):
    ns = tuple(ns)
    if ns not in _prog_cache:
        _prog_cache[ns] = _build_program(ns)
    return _prog_cache[ns]


def _prep_inputs(q, k, v, vl):
    """q,k,v: [n, L, D] fp32; vl: [n] int. Returns (qkm, vp) fp16 arrays."""
    n = q.shape[0]
    qt = np.empty((n, D + 1, L), np.float16)
    qt[:, :D] = (q.transpose(0, 2, 1) * np.float32(0.125)).astype(np.float16)
    qt[:, D] = 1.0
    ktm = np.empty((n, D + 1, L), np.float16)
    ktm[:, :D] = k.transpose(0, 2, 1).astype(np.float16)
    iota = np.arange(L)
    # -1e6 overflows fp16; -60000 is finite in fp16 and still drives both
    # exp paths (ACT underflow, DVE int16 saturation) to exactly 0.
    ktm[:, D] = np.where(iota[None, :] < vl[:, None],
                         np.float16(0.0), np.float16(-60000.0))
    # valid_len == 0: reference softmaxes constant -1e6 -> uniform. Reproduce
    # by zeroing the logits (Q rows and mask row) -> exp(0) everywhere.
    zmask = vl == 0
    if zmask.any():
        qt[zmask, :D] = 0.0
        ktm[zmask, D] = 0.0
    qkm = np.zeros((n, D + 1, QKM_W), np.float16)
    qkm[:, :, :128] = ktm[:, :, :128]
    qkm[:, :, 128:128 + L] = qt
    qkm[:, :, 128 + L:] = ktm[:, :, 128:]
    vpe = np.empty((n, L, D + 1), np.float16)
    vpe[:, :, :D] = v.astype(np.float16)
    vpe[:, :, D] = 1.0
    vpe = vpe.reshape(n, KT, 128, D + 1).transpose(0, 2, 1, 3)
    # pack [vt0 | zeros(195) | vt1 | vt2 | ...] per partition row
    vp = np.zeros((n, 128, 260 + (KT - 1) * (D + 1)), np.float16)
    vp[:, :, :D + 1] = vpe[:, :, 0, :]
    vp[:, :, 260:] = vpe[:, :, 1:, :].reshape(n, 128, (KT - 1) * (D + 1))
    return qkm, vp


def kernel(queries, keys, values, valid_lens):
    queries = np.asarray(queries, np.float32)
    keys = np.asarray(keys, np.float32)
    values = np.asarray(values, np.float32)
    vl = np.asarray(valid_lens).astype(np.int64)

    # Ragged load balancing: sort batches by active k-tile count, deal
    # across cores (slot s <- sorted group) so every core runs the same
    # per-slot tile counts. Slot order (big, small..., big): a big first
    # batch covers the early input-DMA ramp for the small batches queued
    # behind it, and a big last batch leaves only one drain chain exposed.
    nact = np.where(vl == 0, KT, -(-vl // 128)).astype(np.int64)
    order = np.argsort(nact, kind="stable")
    slot_groups = list(range(BPC))
    ns = tuple(int(nact[order[g * N_CORES + N_CORES - 1]])
               for g in slot_groups)

    qkm, vp = _prep_inputs(queries[order], keys[order], values[order],
                           vl[order])

    nc = get_program(ns)
    in_maps = []
    for c in range(N_CORES):
        idx = [slot_groups[s] * N_CORES + c for s in range(BPC)]
        in_maps.append({
            "qkm": np.ascontiguousarray(qkm[idx]),
            "vp": np.ascontiguousarray(vp[idx]),
        })

    res = None
    for attempt in range(3):
        try:
            res = run_bass_kernel_spmd(nc, in_maps, list(range(N_CORES)))
            break
        except Exception:
            # Transient NRT/axon device failures have been observed on the
            # first execution of a freshly compiled NEFF; reset and retry.
            if attempt == 2:
                raise
            import time as _time
            _time.sleep(2.0)
            try:
                import jax
                jax.clear_caches()
            except Exception:
                pass

    out = np.empty((B, L, D), np.float32)
    for c in range(N_CORES):
        raw = res.results[c]["o"][:BPC - 1]  # [BPC-1,128,2,4*65] cumulative
        # slots 0..BPC-2: difference to recover per-slot values; the last
        # slot zero-opened its banks and arrives exact in fp16
        o = np.concatenate([
            raw[:1], np.diff(raw, axis=0),
            res.results[c]["o16"][None].astype(np.float32),
        ])
        o = o.reshape(BPC, 128, 2, 4, D + 1).transpose(0, 2, 3, 1, 4)
        o = o.reshape(BPC, L, D + 1)  # rows [(4h+j)*128 + p]
        on = o[:, :, :D] / o[:, :, D:D + 1]
        for s in range(BPC):
            out[order[slot_groups[s] * N_CORES + c]] = on[s]
    return out


# revision 80
# speedup vs baseline: 1.0513x; 1.0513x over previous
"""Masked dot-product attention (B=64, Lq=Lk=1024, d=64, fp32) on 8 TRN2 cores.

v2 strategy (batch-parallel, 8 batch slots per core), all fp16 operands:
  - Host folds 1/sqrt(d) into Q, converts Q/K/V to fp16, and packs the key
    mask (0 valid / -1e6 masked) as contraction row 64, so masked scores come
    from one matmul: S^T[k, q] = sum_d K[k,d]*Q[q,d]/8 + mask[k]*1.
    fp16 matmuls run at 1 cycle/row for ANY output width (unlike fp32r's
    >=256 requirement), enabling narrow O-matmuls.
  - exp is split across TWO engines, alternating whole k-tiles:
      * ACT: exact exp (PSUM fp32 -> SBUF fp16).
      * DVE: Schraudolph fast-exp in one tensor_scalar:
          bits_i16 = round_sat(s * 1477.3197 + 15315.25)  (= fp16 bits of
          ~exp(s), max rel err 3%). Saturation maps masked -1e6 scores to
          -32768 = fp16 -0.0, i.e. exactly zero attention weight.
  - O accumulates per q-tile: out[128q, 65] += P^T[k, q-tile]^T @ [V|1]
    (65 cols: 64 outputs + softmax denominator). PSUM allows only ONE open
    accumulation group per bank, so the 8 q-groups live at 65-col strides in
    2 banks ([128, 2, 512] tile); each bank's group sequence is opened by a
    single start=True matmul whose rhs is [V|1|zeros] (writes group 0/4's
    data and zeroes the other groups' region), every other matmul
    accumulates with start=False, and the bank's last matmul sets stop=True.
  - DVE compacts [128, 2, 260] PSUM fp32 -> SBUF fp16 for a single
    contiguous output DMA; the divide + final transpose happen on the host.
  - Raggedness: batches sorted by active k-tile count and dealt across cores
    (slot s runs max-of-group tiles on every core); dead k-tiles skipped.
  - Emission is software-pipelined: S-matmul of tile i+1 precedes the
    O-matmuls of tile i so PE never waits on exp.
"""

import numpy as np

import concourse.bass as bass
import concourse.mybir as mybir
import concourse.tile as tile
from concourse import bacc
from concourse.bass_utils import run_bass_kernel_spmd

N_CORES = 8
B = 64
L = 1024
D = 64
BPC = B // N_CORES   # batch slots per core
KT = L // 128        # max k-tiles per batch
NEG_INF = -1000000.0

F16 = mybir.dt.float16
F32 = mybir.dt.float32
I16 = mybir.dt.int16

# Schraudolph fp16 fast-exp constants (C=44.75 minimax-tuned: 3.0% max err)
EXP_A = 1477.3197
EXP_B = 15315.25

QKM_W = 2 * L        # [ktile0 (128) | qt (1024) | ktiles 1..7 (896)]

_prog_cache = {}


def _build_program(ns):
    """ns: per-slot k-tile counts (tuple of BPC ints in 1..KT)."""
    nc = bacc.Bacc("TRN2", target_bir_lowering=False, debug=False,
                   num_devices=N_CORES)
    qkm_d = nc.dram_tensor("qkm", [BPC, D + 1, QKM_W], F16,
                           kind="ExternalInput")
    # vp packs [vt0 | zeros(195) | vt1 | vt2 | ...]: the kt==0 slice is the
    # 260-wide zero-padded rhs of the bank-opening matmuls.
    vp_d = nc.dram_tensor("vp", [BPC, 128, 260 + (KT - 1) * (D + 1)], F16,
                          kind="ExternalInput")
    # fp32: the device ships raw CUMULATIVE accumulator snapshots (batch s's
    # O-matmuls accumulate onto batch s-1's results instead of paying
    # bank-zeroing matmuls); the host recovers per-batch values by
    # differencing, which needs fp32 precision.
    o_d = nc.dram_tensor("o", [BPC, 128, 2, 4 * (D + 1)], F32,
                         kind="ExternalOutput")
    # the last batch zero-opens its banks (no stale base), so its output is
    # exact and ships fp16 -- halving the tail DMA that ends the kernel
    o16_d = nc.dram_tensor("o16", [128, 2, 4 * (D + 1)], F16,
                           kind="ExternalOutput")

    # Flat (batch, ktile) job list for cross-batch software pipelining.
    jobs = [(b, kt) for b in range(BPC) for kt in range(ns[b])]
    n_jobs = len(jobs)
    # Whole-tile exp alternating between ACT (exact) and DVE (Schraudolph):
    # one instruction per tile amortizes the fixed per-instruction engine
    # overhead (~185ns ACT / ~125ns DVE). DVE is ~1.15x slower per tile, so
    # it gets the smaller share, evenly spread. Output copies are split per
    # bank: bank A on ACT, bank B on DVE.
    # DVE is the saturated engine: front-load its tiles (odd ring positions
    # from g=1) so it starts as soon as the second S-matmul lands instead of
    # idling through the pipeline warmup.
    n_dve = round(n_jobs * 0.4865)
    dve_tile = [False] * n_jobs
    if n_dve > 0 and n_jobs > 1:
        for i in range(n_dve):
            dve_tile[1 + (i * (n_jobs - 1)) // n_dve] = True

    with tile.TileContext(nc) as tc:
        with (
            tc.tile_pool(name="qkm", bufs=3) as qkm_pool,
            tc.tile_pool(name="vpp", bufs=3) as vp_pool,
            tc.tile_pool(name="pt", bufs=7) as pt_pool,
            tc.tile_pool(name="osb", bufs=3) as osb_pool,
            tc.tile_pool(name="sp", bufs=3, space="PSUM") as sp_pool,
            tc.tile_pool(name="op", bufs=1, space="PSUM") as op_pool,
        ):
            qkm_s = [None] * BPC
            vp_s = [None] * BPC
            opsum = [None] * BPC
            pt_of_job = [None] * n_jobs
            started = [False] * (BPC + 1)

            # warm the ACT exp table (1.3us load) during the DMA-latency head
            sc = qkm_pool.tile([128, 1], F32, tag="sc", name="sc")
            nc.gpsimd.memset(sc[:], 0.0)
            nc.scalar.activation(sc[:], sc[:],
                                 mybir.ActivationFunctionType.Exp)

            def ktm_sl(b, kt):
                if kt == 0:
                    return qkm_s[b][:, :128]
                o = 128 + L + (kt - 1) * 128
                return qkm_s[b][:, o:o + 128]

            def start_batch(b):
                nkt = ns[b]
                end = 128 + L + (nkt - 1) * 128
                vend = 260 + (nkt - 1) * (D + 1)
                qkm = qkm_pool.tile([D + 1, QKM_W], F16, tag="qkm",
                                    name=f"qkm{b}")
                vp = vp_pool.tile([128, 260 + (KT - 1) * (D + 1)], F16,
                                  tag="vp", name=f"vp{b}")
                if b == 0:
                    # head: ktile0 + first half of qt -> first matmuls sooner
                    nc.sync.dma_start(qkm[:, :640], qkm_d[b][:, :640])
                    nc.sync.dma_start(qkm[:, 640:end], qkm_d[b][:, 640:end])
                else:
                    nc.sync.dma_start(qkm[:, :end], qkm_d[b][:, :end])
                nc.sync.dma_start(vp[:, :vend], vp_d[b][:, :vend])
                qkm_s[b] = qkm
                vp_s[b] = vp

            def ensure_started(b):
                if b < BPC and not started[b]:
                    started[b] = True
                    start_batch(b)

            def s_and_exp(g):
                b, kt = jobs[g]
                ensure_started(b)
                # prefetch inputs for every batch reached within 6 jobs
                tb = jobs[min(g + 6, n_jobs - 1)][0]
                for bb in range(b + 1, tb + 1):
                    ensure_started(bb)
                nkt = ns[b]
                qt = qkm_s[b][:, 128:128 + L]
                sp = sp_pool.tile([128, L], F32, tag="sp", name=f"sp{g}")
                pt = pt_pool.tile([128, L], F16, tag="pt", name=f"pt{g}")
                pt_of_job[g] = pt
                if g == 0:
                    # quarter-width matmuls to prime the pipeline
                    for qo in range(0, L, 256):
                        nc.tensor.matmul(sp[:, qo:qo + 256], ktm_sl(b, kt),
                                         qt[:, qo:qo + 256],
                                         start=True, stop=True)
                else:
                    for qo in range(0, L, 512):
                        nc.tensor.matmul(sp[:, qo:qo + 512], ktm_sl(b, kt),
                                         qt[:, qo:qo + 512],
                                         start=True, stop=True)
                if dve_tile[g] and g > 0:
                    nc.vector.tensor_scalar(
                        pt.bitcast(I16)[:], sp[:], EXP_A, EXP_B,
                        mybir.AluOpType.mult, mybir.AluOpType.add)
                elif g == 0:
                    # split exp so the first O-matmuls can start sooner
                    for eo in range(0, L, 512):
                        nc.scalar.activation(
                            pt[:, eo:eo + 512], sp[:, eo:eo + 512],
                            mybir.ActivationFunctionType.Exp)
                else:
                    nc.scalar.activation(pt[:], sp[:],
                                         mybir.ActivationFunctionType.Exp)

            def o_mm(g):
                b, kt = jobs[g]
                nkt = ns[b]
                if kt == 0:
                    # two independent 1-bank accumulator lines so batch b+1's
                    # bank-h accumulate waits only on bank-h's copy of batch b
                    opsum[b] = [
                        op_pool.tile([128, 512], F32, tag=f"oph{h}",
                                     name=f"op{b}h{h}")
                        for h in range(2)
                    ]
                pt = pt_of_job[g]
                last = kt == nkt - 1
                if kt == 0 and (b == 0 or b == BPC - 1):
                    # PSUM is undefined at program start (b==0), and the
                    # last batch opts out of the stale-accumulate chain so
                    # its output is exact fp16: open each bank with a
                    # zero-padded matmul (group 4h's data + zeroed region).
                    # All other batches accumulate onto the previous batch's
                    # (already copied-out) results; the host undoes the
                    # running sum by differencing.
                    vz = vp_s[b][:, :260]  # [V|1|zeros(195)]
                    for h in range(2):
                        nc.tensor.matmul(
                            opsum[b][h][:, :260],
                            pt[:, h * 512:h * 512 + 128], vz,
                            start=True, stop=False,
                            skip_group_check=True)
                    qlist = [1, 2, 3, 5, 6, 7]
                else:
                    qlist = list(range(8))
                vt = vp_s[b][:, kt * (D + 1) + 195:kt * (D + 1) + 260] \
                    if kt > 0 else vp_s[b][:, :D + 1]
                for q in qlist:
                    h, j = divmod(q, 4)
                    nc.tensor.matmul(
                        opsum[b][h][:, j * (D + 1):(j + 1) * (D + 1)],
                        pt[:, q * 128:(q + 1) * 128],
                        vt,
                        start=False,
                        stop=last and (j == 3),
                        skip_group_check=True,
                    )
                pt_of_job[g] = None
                if last and b == BPC - 1:
                    # tail: per-bank copies in parallel (ACT || DVE), fp16
                    osbL = osb_pool.tile([128, 2, 4 * (D + 1)], F16,
                                         tag="osbL", name="osbL")
                    nc.scalar.copy(osbL[:, 0, :], opsum[b][0][:, :4 * (D + 1)])
                    nc.vector.tensor_copy(osbL[:, 1, :],
                                          opsum[b][1][:, :4 * (D + 1)])
                    nc.sync.dma_start(o16_d[:], osbL[:])
                elif last:
                    osb = osb_pool.tile([128, 2, 4 * (D + 1)], F32,
                                        tag="osb", name=f"osb{b}")
                    nc.scalar.copy(osb[:, 0, :], opsum[b][0][:, :4 * (D + 1)])
                    nc.vector.tensor_copy(osb[:, 1, :],
                                          opsum[b][1][:, :4 * (D + 1)])
                    # one DMA per batch, alternating queues (Pool SWDGE / SP)
                    # so neither queue's serial ~1-2us per DMA piles up into
                    # an end-of-kernel drain tail
                    if b % 2 == 0:
                        nc.gpsimd.dma_start(o_d[b], osb[:])
                    else:
                        nc.sync.dma_start(o_d[b], osb[:])

            # pipeline: O of job g trails the S/exp stream by LOOKAHEAD jobs
            # so the PE wait-queue never blocks on an in-flight exp. O (and
            # the trailing copies) are emitted BEFORE the S/exp at the same
            # position so copies land ahead of exp in the ACT/DVE queues.
            LOOKAHEAD = 5
            for g in range(n_jobs):
                if g >= LOOKAHEAD:
                    o_mm(g - LOOKAHEAD)
                s_and_exp(g)
            for g in range(max(0, n_jobs - LOOKAHEAD), n_jobs):
                o_mm(g)

    nc.compile()
    return nc


def get_program(ns):
    ns = tuple(ns)
    if ns not in _prog_cache:
        _prog_cache[ns] = _build_program(ns)
    return _prog_cache[ns]


def _prep_inputs(q, k, v, vl):
    """q,k,v: [n, L, D] fp32; vl: [n] int. Returns (qkm, vp) fp16 arrays."""
    n = q.shape[0]
    qt = np.empty((n, D + 1, L), np.float16)
    qt[:, :D] = (q.transpose(0, 2, 1) * np.float32(0.125)).astype(np.float16)
    qt[:, D] = 1.0
    ktm = np.empty((n, D + 1, L), np.float16)
    ktm[:, :D] = k.transpose(0, 2, 1).astype(np.float16)
    iota = np.arange(L)
    # -1e6 overflows fp16; -60000 is finite in fp16 and still drives both
    # exp paths (ACT underflow, DVE int16 saturation) to exactly 0.
    ktm[:, D] = np.where(iota[None, :] < vl[:, None],
                         np.float16(0.0), np.float16(-60000.0))
    # valid_len == 0: reference softmaxes constant -1e6 -> uniform. Reproduce
    # by zeroing the logits (Q rows and mask row) -> exp(0) everywhere.
    zmask = vl == 0
    if zmask.any():
        qt[zmask, :D] = 0.0
        ktm[zmask, D] = 0.0
    qkm = np.zeros((n, D + 1, QKM_W), np.float16)
    qkm[:, :, :128] = ktm[:, :, :128]
    qkm[:, :, 128:128 + L] = qt
    qkm[:, :, 128 + L:] = ktm[:, :, 128:]
    vpe = np.empty((n, L, D + 1), np.float16)
    vpe[:, :, :D] = v.astype(np.float16)
    vpe[:, :, D] = 1.0
    vpe = vpe.reshape(n, KT, 128, D + 1).transpose(0, 2, 1, 3)
    # pack [vt0 | zeros(195) | vt1 | vt2 | ...] per partition row
    vp = np.zeros((n, 128, 260 + (KT - 1) * (D + 1)), np.float16)
    vp[:, :, :D + 1] = vpe[:, :, 0, :]
    vp[:, :, 260:] = vpe[:, :, 1:, :].reshape(n, 128, (KT - 1) * (D + 1))
    return qkm, vp


def kernel(queries, keys, values, valid_lens):
    queries = np.asarray(queries, np.float32)
    keys = np.asarray(keys, np.float32)
    values = np.asarray(values, np.float32)
    vl = np.asarray(valid_lens).astype(np.int64)

    # Ragged load balancing: sort batches by active k-tile count, deal
    # across cores (slot s <- sorted group) so every core runs the same
    # per-slot tile counts. Slot order (big, small..., big): a big first
    # batch covers the early input-DMA ramp for the small batches queued
    # behind it, and a big last batch leaves only one drain chain exposed.
    nact = np.where(vl == 0, KT, -(-vl // 128)).astype(np.int64)
    order = np.argsort(nact, kind="stable")
    slot_groups = list(range(BPC))
    ns = tuple(int(nact[order[g * N_CORES + N_CORES - 1]])
               for g in slot_groups)

    qkm, vp = _prep_inputs(queries[order], keys[order], values[order],
                           vl[order])

    nc = get_program(ns)
    in_maps = []
    for c in range(N_CORES):
        idx = [slot_groups[s] * N_CORES + c for s in range(BPC)]
        in_maps.append({
            "qkm": np.ascontiguousarray(qkm[idx]),
            "vp": np.ascontiguousarray(vp[idx]),
        })

    res = None
    for attempt in range(3):
        try:
            res = run_bass_kernel_spmd(nc, in_maps, list(range(N_CORES)))
            break
        except Exception:
            # Transient NRT/axon device failures have been observed on the
            # first execution of a freshly compiled NEFF; reset and retry.
            if attempt == 2:
                raise
            import time as _time
            _time.sleep(2.0)
            try:
                import jax
                jax.clear_caches()
            except Exception:
                pass

    out = np.empty((B, L, D), np.float32)
    for c in range(N_CORES):
        raw = res.results[c]["o"][:BPC - 1]  # [BPC-1,128,2,4*65] cumulative
        # slots 0..BPC-2: difference to recover per-slot values; the last
        # slot zero-opened its banks and arrives exact in fp16
        o = np.concatenate([
            raw[:1], np.diff(raw, axis=0),
            res.results[c]["o16"][None].astype(np.float32),
        ])
        o = o.reshape(BPC, 128, 2, 4, D + 1).transpose(0, 2, 3, 1, 4)
        o = o.reshape(BPC, L, D + 1)  # rows [(4h+j)*128 + p]
        on = o[:, :, :D] / o[:, :, D:D + 1]
        for s in range(BPC):
            out[order[slot_groups[s] * N_CORES + c]] = on[s]
    return out


# revision 81
# speedup vs baseline: 1.0551x; 1.0037x over previous
"""Masked dot-product attention (B=64, Lq=Lk=1024, d=64, fp32) on 8 TRN2 cores.

v2 strategy (batch-parallel, 8 batch slots per core), all fp16 operands:
  - Host folds 1/sqrt(d) into Q, converts Q/K/V to fp16, and packs the key
    mask (0 valid / -1e6 masked) as contraction row 64, so masked scores come
    from one matmul: S^T[k, q] = sum_d K[k,d]*Q[q,d]/8 + mask[k]*1.
    fp16 matmuls run at 1 cycle/row for ANY output width (unlike fp32r's
    >=256 requirement), enabling narrow O-matmuls.
  - exp is split across TWO engines, alternating whole k-tiles:
      * ACT: exact exp (PSUM fp32 -> SBUF fp16).
      * DVE: Schraudolph fast-exp in one tensor_scalar:
          bits_i16 = round_sat(s * 1477.3197 + 15315.25)  (= fp16 bits of
          ~exp(s), max rel err 3%). Saturation maps masked -1e6 scores to
          -32768 = fp16 -0.0, i.e. exactly zero attention weight.
  - O accumulates per q-tile: out[128q, 65] += P^T[k, q-tile]^T @ [V|1]
    (65 cols: 64 outputs + softmax denominator). PSUM allows only ONE open
    accumulation group per bank, so the 8 q-groups live at 65-col strides in
    2 banks ([128, 2, 512] tile); each bank's group sequence is opened by a
    single start=True matmul whose rhs is [V|1|zeros] (writes group 0/4's
    data and zeroes the other groups' region), every other matmul
    accumulates with start=False, and the bank's last matmul sets stop=True.
  - DVE compacts [128, 2, 260] PSUM fp32 -> SBUF fp16 for a single
    contiguous output DMA; the divide + final transpose happen on the host.
  - Raggedness: batches sorted by active k-tile count and dealt across cores
    (slot s runs max-of-group tiles on every core); dead k-tiles skipped.
  - Emission is software-pipelined: S-matmul of tile i+1 precedes the
    O-matmuls of tile i so PE never waits on exp.
"""

import numpy as np

import concourse.bass as bass
import concourse.mybir as mybir
import concourse.tile as tile
from concourse import bacc
from concourse.bass_utils import run_bass_kernel_spmd

N_CORES = 8
B = 64
L = 1024
D = 64
BPC = B // N_CORES   # batch slots per core
KT = L // 128        # max k-tiles per batch
NEG_INF = -1000000.0

F16 = mybir.dt.float16
F32 = mybir.dt.float32
I16 = mybir.dt.int16

# Schraudolph fp16 fast-exp constants (C=44.75 minimax-tuned: 3.0% max err)
EXP_A = 1477.3197
EXP_B = 15315.25

QKM_W = 2 * L        # [ktile0 (128) | qt (1024) | ktiles 1..7 (896)]

_prog_cache = {}


def _build_program(ns):
    """ns: per-slot k-tile counts (tuple of BPC ints in 1..KT)."""
    nc = bacc.Bacc("TRN2", target_bir_lowering=False, debug=False,
                   num_devices=N_CORES)
    qkm_d = nc.dram_tensor("qkm", [BPC, D + 1, QKM_W], F16,
                           kind="ExternalInput")
    # vp packs [vt0 | zeros(195) | vt1 | vt2 | ...]: the kt==0 slice is the
    # 260-wide zero-padded rhs of the bank-opening matmuls.
    vp_d = nc.dram_tensor("vp", [BPC, 128, 260 + (KT - 1) * (D + 1)], F16,
                          kind="ExternalInput")
    # fp32: the device ships raw CUMULATIVE accumulator snapshots (batch s's
    # O-matmuls accumulate onto batch s-1's results instead of paying
    # bank-zeroing matmuls); the host recovers per-batch values by
    # differencing, which needs fp32 precision.
    o_d = nc.dram_tensor("o", [BPC, 128, 2, 4 * (D + 1)], F32,
                         kind="ExternalOutput")
    # the last batch zero-opens its banks (no stale base), so its output is
    # exact and ships fp16 -- halving the tail DMA that ends the kernel
    o16_d = nc.dram_tensor("o16", [128, 2, 4 * (D + 1)], F16,
                           kind="ExternalOutput")

    # Flat (batch, ktile) job list for cross-batch software pipelining.
    jobs = [(b, kt) for b in range(BPC) for kt in range(ns[b])]
    n_jobs = len(jobs)
    # Whole-tile exp alternating between ACT (exact) and DVE (Schraudolph):
    # one instruction per tile amortizes the fixed per-instruction engine
    # overhead (~185ns ACT / ~125ns DVE). DVE is ~1.15x slower per tile, so
    # it gets the smaller share, evenly spread. Output copies are split per
    # bank: bank A on ACT, bank B on DVE.
    # DVE is the saturated engine: front-load its tiles (odd ring positions
    # from g=1) so it starts as soon as the second S-matmul lands instead of
    # idling through the pipeline warmup.
    n_dve = round(n_jobs * 0.4865)
    dve_tile = [False] * n_jobs
    if n_dve > 0 and n_jobs > 1:
        for i in range(n_dve):
            dve_tile[1 + (i * (n_jobs - 1)) // n_dve] = True

    with tile.TileContext(nc) as tc:
        with (
            tc.tile_pool(name="qkm", bufs=3) as qkm_pool,
            tc.tile_pool(name="vpp", bufs=3) as vp_pool,
            tc.tile_pool(name="pt", bufs=7) as pt_pool,
            tc.tile_pool(name="osb", bufs=3) as osb_pool,
            tc.tile_pool(name="sp", bufs=3, space="PSUM") as sp_pool,
            tc.tile_pool(name="op", bufs=1, space="PSUM") as op_pool,
        ):
            qkm_s = [None] * BPC
            vp_s = [None] * BPC
            opsum = [None] * BPC
            pt_of_job = [None] * n_jobs
            started = [False] * (BPC + 1)

            # warm the ACT exp table (1.3us load) during the DMA-latency head
            sc = qkm_pool.tile([128, 1], F32, tag="sc", name="sc")
            nc.gpsimd.memset(sc[:], 0.0)
            nc.scalar.activation(sc[:], sc[:],
                                 mybir.ActivationFunctionType.Exp)

            def ktm_sl(b, kt):
                if kt == 0:
                    return qkm_s[b][:, :128]
                o = 128 + L + (kt - 1) * 128
                return qkm_s[b][:, o:o + 128]

            def start_batch(b):
                nkt = ns[b]
                end = 128 + L + (nkt - 1) * 128
                vend = 260 + (nkt - 1) * (D + 1)
                qkm = qkm_pool.tile([D + 1, QKM_W], F16, tag="qkm",
                                    name=f"qkm{b}")
                vp = vp_pool.tile([128, 260 + (KT - 1) * (D + 1)], F16,
                                  tag="vp", name=f"vp{b}")
                if b == 0:
                    # head: ktile0 + first half of qt -> first matmuls sooner
                    nc.sync.dma_start(qkm[:, :640], qkm_d[b][:, :640])
                    nc.sync.dma_start(qkm[:, 640:end], qkm_d[b][:, 640:end])
                else:
                    nc.sync.dma_start(qkm[:, :end], qkm_d[b][:, :end])
                nc.sync.dma_start(vp[:, :vend], vp_d[b][:, :vend])
                qkm_s[b] = qkm
                vp_s[b] = vp

            def ensure_started(b):
                if b < BPC and not started[b]:
                    started[b] = True
                    start_batch(b)

            def s_and_exp(g):
                b, kt = jobs[g]
                ensure_started(b)
                # prefetch inputs for every batch reached within 6 jobs
                tb = jobs[min(g + 6, n_jobs - 1)][0]
                for bb in range(b + 1, tb + 1):
                    ensure_started(bb)
                nkt = ns[b]
                qt = qkm_s[b][:, 128:128 + L]
                sp = sp_pool.tile([128, L], F32, tag="sp", name=f"sp{g}")
                pt = pt_pool.tile([128, L], F16, tag="pt", name=f"pt{g}")
                pt_of_job[g] = pt
                if g == 0:
                    # quarter-width matmuls to prime the pipeline
                    for qo in range(0, L, 256):
                        nc.tensor.matmul(sp[:, qo:qo + 256], ktm_sl(b, kt),
                                         qt[:, qo:qo + 256],
                                         start=True, stop=True)
                else:
                    for qo in range(0, L, 512):
                        nc.tensor.matmul(sp[:, qo:qo + 512], ktm_sl(b, kt),
                                         qt[:, qo:qo + 512],
                                         start=True, stop=True)
                if dve_tile[g] and g > 0:
                    nc.vector.tensor_scalar(
                        pt.bitcast(I16)[:], sp[:], EXP_A, EXP_B,
                        mybir.AluOpType.mult, mybir.AluOpType.add)
                elif g == 0:
                    # split exp so the first O-matmuls can start sooner
                    for eo in range(0, L, 512):
                        nc.scalar.activation(
                            pt[:, eo:eo + 512], sp[:, eo:eo + 512],
                            mybir.ActivationFunctionType.Exp)
                else:
                    nc.scalar.activation(pt[:], sp[:],
                                         mybir.ActivationFunctionType.Exp)

            def o_mm(g):
                b, kt = jobs[g]
                nkt = ns[b]
                if kt == 0:
                    # two independent 1-bank accumulator lines so batch b+1's
                    # bank-h accumulate waits only on bank-h's copy of batch b
                    opsum[b] = [
                        op_pool.tile([128, 512], F32, tag=f"oph{h}",
                                     name=f"op{b}h{h}")
                        for h in range(2)
                    ]
                pt = pt_of_job[g]
                last = kt == nkt - 1
                if kt == 0 and (b == 0 or b == BPC - 1):
                    # PSUM is undefined at program start (b==0), and the
                    # last batch opts out of the stale-accumulate chain so
                    # its output is exact fp16: open each bank with a
                    # zero-padded matmul (group 4h's data + zeroed region).
                    # All other batches accumulate onto the previous batch's
                    # (already copied-out) results; the host undoes the
                    # running sum by differencing.
                    vz = vp_s[b][:, :260]  # [V|1|zeros(195)]
                    for h in range(2):
                        nc.tensor.matmul(
                            opsum[b][h][:, :260],
                            pt[:, h * 512:h * 512 + 128], vz,
                            start=True, stop=False,
                            skip_group_check=True)
                    qlist = [1, 2, 3, 5, 6, 7]
                else:
                    qlist = list(range(8))
                vt = vp_s[b][:, kt * (D + 1) + 195:kt * (D + 1) + 260] \
                    if kt > 0 else vp_s[b][:, :D + 1]
                for q in qlist:
                    h, j = divmod(q, 4)
                    nc.tensor.matmul(
                        opsum[b][h][:, j * (D + 1):(j + 1) * (D + 1)],
                        pt[:, q * 128:(q + 1) * 128],
                        vt,
                        start=False,
                        stop=last and (j == 3),
                        skip_group_check=True,
                    )
                pt_of_job[g] = None
                if last and b == BPC - 1:
                    # tail: per-bank copies in parallel (ACT || DVE), fp16
                    osbL = osb_pool.tile([128, 2, 4 * (D + 1)], F16,
                                         tag="osbL", name="osbL")
                    nc.scalar.copy(osbL[:, 0, :], opsum[b][0][:, :4 * (D + 1)])
                    nc.vector.tensor_copy(osbL[:, 1, :],
                                          opsum[b][1][:, :4 * (D + 1)])
                    nc.sync.dma_start(o16_d[:], osbL[:])
                elif last:
                    osb = osb_pool.tile([128, 2, 4 * (D + 1)], F32,
                                        tag="osb", name=f"osb{b}")
                    nc.scalar.copy(osb[:, 0, :], opsum[b][0][:, :4 * (D + 1)])
                    nc.vector.tensor_copy(osb[:, 1, :],
                                          opsum[b][1][:, :4 * (D + 1)])
                    # one DMA per batch, alternating queues (Pool SWDGE / SP)
                    # so neither queue's serial ~1-2us per DMA piles up into
                    # an end-of-kernel drain tail
                    if b % 2 == 0:
                        nc.gpsimd.dma_start(o_d[b], osb[:])
                    else:
                        nc.sync.dma_start(o_d[b], osb[:])

            # pipeline: O of job g trails the S/exp stream by LOOKAHEAD jobs
            # so the PE wait-queue never blocks on an in-flight exp. O (and
            # the trailing copies) are emitted BEFORE the S/exp at the same
            # position so copies land ahead of exp in the ACT/DVE queues.
            LOOKAHEAD = 5
            for g in range(n_jobs):
                if g >= LOOKAHEAD:
                    o_mm(g - LOOKAHEAD)
                s_and_exp(g)
            for g in range(max(0, n_jobs - LOOKAHEAD), n_jobs):
                o_mm(g)

    nc.compile()
    return nc


def get_program(ns):
    ns = tuple(ns)
    if ns not in _prog_cache:
        _prog_cache[ns] = _build_program(ns)
    return _prog_cache[ns]


def _prep_inputs(q, k, v, vl):
    """q,k,v: [n, L, D] fp32; vl: [n] int. Returns (qkm, vp) fp16 arrays."""
    n = q.shape[0]
    qt = np.empty((n, D + 1, L), np.float16)
    qt[:, :D] = (q.transpose(0, 2, 1) * np.float32(0.125)).astype(np.float16)
    qt[:, D] = 1.0
    ktm = np.empty((n, D + 1, L), np.float16)
    ktm[:, :D] = k.transpose(0, 2, 1).astype(np.float16)
    iota = np.arange(L)
    # -1e6 overflows fp16; -60000 is finite in fp16 and still drives both
    # exp paths (ACT underflow, DVE int16 saturation) to exactly 0.
    ktm[:, D] = np.where(iota[None, :] < vl[:, None],
                         np.float16(0.0), np.float16(-60000.0))
    # valid_len == 0: reference softmaxes constant -1e6 -> uniform. Reproduce
    # by zeroing the logits (Q rows and mask row) -> exp(0) everywhere.
    zmask = vl == 0
    if zmask.any():
        qt[zmask, :D] = 0.0
        ktm[zmask, D] = 0.0
    qkm = np.zeros((n, D + 1, QKM_W), np.float16)
    qkm[:, :, :128] = ktm[:, :, :128]
    qkm[:, :, 128:128 + L] = qt
    qkm[:, :, 128 + L:] = ktm[:, :, 128:]
    vpe = np.empty((n, L, D + 1), np.float16)
    vpe[:, :, :D] = v.astype(np.float16)
    vpe[:, :, D] = 1.0
    vpe = vpe.reshape(n, KT, 128, D + 1).transpose(0, 2, 1, 3)
    # pack [vt0 | zeros(195) | vt1 | vt2 | ...] per partition row
    vp = np.zeros((n, 128, 260 + (KT - 1) * (D + 1)), np.float16)
    vp[:, :, :D + 1] = vpe[:, :, 0, :]
    vp[:, :, 260:] = vpe[:, :, 1:, :].reshape(n, 128, (KT - 1) * (D + 1))
    return qkm, vp


def kernel(queries, keys, values, valid_lens):
    queries = np.asarray(queries, np.float32)
    keys = np.asarray(keys, np.float32)
    values = np.asarray(values, np.float32)
    vl = np.asarray(valid_lens).astype(np.int64)

    # Ragged load balancing: sort batches by active k-tile count, deal
    # across cores (slot s <- sorted group) so every core runs the same
    # per-slot tile counts. Slot order (big, small..., big): a big first
    # batch covers the early input-DMA ramp for the small batches queued
    # behind it, and a big last batch leaves only one drain chain exposed.
    nact = np.where(vl == 0, KT, -(-vl // 128)).astype(np.int64)
    order = np.argsort(nact, kind="stable")
    # big batch first (covers the early input-DMA ramp so the saturated DVE
    # engine starts sooner), small batches mid, big batch last (one exposed
    # drain chain)
    slot_groups = [BPC - 2] + list(range(BPC - 2)) + [BPC - 1]
    ns = tuple(int(nact[order[g * N_CORES + N_CORES - 1]])
               for g in slot_groups)

    qkm, vp = _prep_inputs(queries[order], keys[order], values[order],
                           vl[order])

    nc = get_program(ns)
    in_maps = []
    for c in range(N_CORES):
        idx = [slot_groups[s] * N_CORES + c for s in range(BPC)]
        in_maps.append({
            "qkm": np.ascontiguousarray(qkm[idx]),
            "vp": np.ascontiguousarray(vp[idx]),
        })

    res = None
    for attempt in range(3):
        try:
            res = run_bass_kernel_spmd(nc, in_maps, list(range(N_CORES)))
            break
        except Exception:
            # Transient NRT/axon device failures have been observed on the
            # first execution of a freshly compiled NEFF; reset and retry.
            if attempt == 2:
                raise
            import time as _time
            _time.sleep(2.0)
            try:
                import jax
                jax.clear_caches()
            except Exception:
                pass

    out = np.empty((B, L, D), np.float32)
    for c in range(N_CORES):
        raw = res.results[c]["o"][:BPC - 1]  # [BPC-1,128,2,4*65] cumulative
        # slots 0..BPC-2: difference to recover per-slot values; the last
        # slot zero-opened its banks and arrives exact in fp16
        o = np.concatenate([
            raw[:1], np.diff(raw, axis=0),
            res.results[c]["o16"][None].astype(np.float32),
        ])
        o = o.reshape(BPC, 128, 2, 4, D + 1).transpose(0, 2, 3, 1, 4)
        o = o.reshape(BPC, L, D + 1)  # rows [(4h+j)*128 + p]
        on = o[:, :, :D] / o[:, :, D:D + 1]
        for s in range(BPC):
            out[order[slot_groups[s] * N_CORES + c]] = on[s]
    return out


# revision 82
# speedup vs baseline: 1.0647x; 1.0090x over previous
"""Masked dot-product attention (B=64, Lq=Lk=1024, d=64, fp32) on 8 TRN2 cores.

v2 strategy (batch-parallel, 8 batch slots per core), all fp16 operands:
  - Host folds 1/sqrt(d) into Q, converts Q/K/V to fp16, and packs the key
    mask (0 valid / -1e6 masked) as contraction row 64, so masked scores come
    from one matmul: S^T[k, q] = sum_d K[k,d]*Q[q,d]/8 + mask[k]*1.
    fp16 matmuls run at 1 cycle/row for ANY output width (unlike fp32r's
    >=256 requirement), enabling narrow O-matmuls.
  - exp is split across TWO engines, alternating whole k-tiles:
      * ACT: exact exp (PSUM fp32 -> SBUF fp16).
      * DVE: Schraudolph fast-exp in one tensor_scalar:
          bits_i16 = round_sat(s * 1477.3197 + 15315.25)  (= fp16 bits of
          ~exp(s), max rel err 3%). Saturation maps masked -1e6 scores to
          -32768 = fp16 -0.0, i.e. exactly zero attention weight.
  - O accumulates per q-tile: out[128q, 65] += P^T[k, q-tile]^T @ [V|1]
    (65 cols: 64 outputs + softmax denominator). PSUM allows only ONE open
    accumulation group per bank, so the 8 q-groups live at 65-col strides in
    2 banks ([128, 2, 512] tile); each bank's group sequence is opened by a
    single start=True matmul whose rhs is [V|1|zeros] (writes group 0/4's
    data and zeroes the other groups' region), every other matmul
    accumulates with start=False, and the bank's last matmul sets stop=True.
  - DVE compacts [128, 2, 260] PSUM fp32 -> SBUF fp16 for a single
    contiguous output DMA; the divide + final transpose happen on the host.
  - Raggedness: batches sorted by active k-tile count and dealt across cores
    (slot s runs max-of-group tiles on every core); dead k-tiles skipped.
  - Emission is software-pipelined: S-matmul of tile i+1 precedes the
    O-matmuls of tile i so PE never waits on exp.
"""

import numpy as np

import concourse.bass as bass
import concourse.mybir as mybir
import concourse.tile as tile
from concourse import bacc
from concourse.bass_utils import run_bass_kernel_spmd

N_CORES = 8
B = 64
L = 1024
D = 64
BPC = B // N_CORES   # batch slots per core
KT = L // 128        # max k-tiles per batch
NEG_INF = -1000000.0

F16 = mybir.dt.float16
F32 = mybir.dt.float32
I16 = mybir.dt.int16

# Schraudolph fp16 fast-exp constants (C=44.75 minimax-tuned: 3.0% max err)
EXP_A = 1477.3197
EXP_B = 15315.25

QKM_W = 2 * L        # [ktile0 (128) | qt (1024) | ktiles 1..7 (896)]

_prog_cache = {}


def _build_program(ns):
    """ns: per-slot k-tile counts (tuple of BPC ints in 1..KT)."""
    nc = bacc.Bacc("TRN2", target_bir_lowering=False, debug=False,
                   num_devices=N_CORES)
    qkm_d = nc.dram_tensor("qkm", [BPC, D + 1, QKM_W], F16,
                           kind="ExternalInput")
    # vp packs [vt0 | zeros(195) | vt1 | vt2 | ...]: the kt==0 slice is the
    # 260-wide zero-padded rhs of the bank-opening matmuls.
    vp_d = nc.dram_tensor("vp", [BPC, 128, 260 + (KT - 1) * (D + 1)], F16,
                          kind="ExternalInput")
    # fp32: the device ships raw CUMULATIVE accumulator snapshots (batch s's
    # O-matmuls accumulate onto batch s-1's results instead of paying
    # bank-zeroing matmuls); the host recovers per-batch values by
    # differencing, which needs fp32 precision.
    o_d = nc.dram_tensor("o", [BPC, 128, 2, 4 * (D + 1)], F32,
                         kind="ExternalOutput")
    # the last batch zero-opens its banks (no stale base), so its output is
    # exact and ships fp16 -- halving the tail DMA that ends the kernel
    o16_d = nc.dram_tensor("o16", [128, 2, 4 * (D + 1)], F16,
                           kind="ExternalOutput")

    # Flat (batch, ktile) job list for cross-batch software pipelining.
    jobs = [(b, kt) for b in range(BPC) for kt in range(ns[b])]
    n_jobs = len(jobs)
    # Whole-tile exp alternating between ACT (exact) and DVE (Schraudolph):
    # one instruction per tile amortizes the fixed per-instruction engine
    # overhead (~185ns ACT / ~125ns DVE). DVE is ~1.15x slower per tile, so
    # it gets the smaller share, evenly spread. Output copies are split per
    # bank: bank A on ACT, bank B on DVE.
    # DVE is the saturated engine: front-load its tiles (odd ring positions
    # from g=1) so it starts as soon as the second S-matmul lands instead of
    # idling through the pipeline warmup.
    n_dve = round(n_jobs * 0.4865)
    dve_tile = [False] * n_jobs
    if n_dve > 0 and n_jobs > 1:
        for i in range(n_dve):
            dve_tile[1 + (i * (n_jobs - 1)) // n_dve] = True

    with tile.TileContext(nc) as tc:
        with (
            tc.tile_pool(name="qkm", bufs=3) as qkm_pool,
            tc.tile_pool(name="vpp", bufs=3) as vp_pool,
            tc.tile_pool(name="pt", bufs=7) as pt_pool,
            tc.tile_pool(name="osb", bufs=3) as osb_pool,
            tc.tile_pool(name="sp", bufs=3, space="PSUM") as sp_pool,
            tc.tile_pool(name="op", bufs=1, space="PSUM") as op_pool,
        ):
            qkm_s = [None] * BPC
            vp_s = [None] * BPC
            opsum = [None] * BPC
            pt_of_job = [None] * n_jobs
            started = [False] * (BPC + 1)

            # warm the ACT exp table (1.3us load) during the DMA-latency head
            sc = qkm_pool.tile([128, 1], F32, tag="sc", name="sc")
            nc.gpsimd.memset(sc[:], 0.0)
            nc.scalar.activation(sc[:], sc[:],
                                 mybir.ActivationFunctionType.Exp)

            def ktm_sl(b, kt):
                if kt == 0:
                    return qkm_s[b][:, :128]
                o = 128 + L + (kt - 1) * 128
                return qkm_s[b][:, o:o + 128]

            def start_batch(b):
                nkt = ns[b]
                end = 128 + L + (nkt - 1) * 128
                vend = 260 + (nkt - 1) * (D + 1)
                qkm = qkm_pool.tile([D + 1, QKM_W], F16, tag="qkm",
                                    name=f"qkm{b}")
                vp = vp_pool.tile([128, 260 + (KT - 1) * (D + 1)], F16,
                                  tag="vp", name=f"vp{b}")
                if b == 0:
                    # head: ktile0 + the FULL qt so tile 0's entire S-matmul
                    # set runs without waiting on the second DMA chain
                    nc.sync.dma_start(qkm[:, :1152], qkm_d[b][:, :1152])
                    if end > 1152:
                        nc.sync.dma_start(qkm[:, 1152:end],
                                          qkm_d[b][:, 1152:end])
                else:
                    nc.sync.dma_start(qkm[:, :end], qkm_d[b][:, :end])
                nc.sync.dma_start(vp[:, :vend], vp_d[b][:, :vend])
                qkm_s[b] = qkm
                vp_s[b] = vp

            def ensure_started(b):
                if b < BPC and not started[b]:
                    started[b] = True
                    start_batch(b)

            def s_and_exp(g):
                b, kt = jobs[g]
                ensure_started(b)
                # prefetch inputs for every batch reached within 6 jobs
                tb = jobs[min(g + 6, n_jobs - 1)][0]
                for bb in range(b + 1, tb + 1):
                    ensure_started(bb)
                nkt = ns[b]
                qt = qkm_s[b][:, 128:128 + L]
                sp = sp_pool.tile([128, L], F32, tag="sp", name=f"sp{g}")
                pt = pt_pool.tile([128, L], F16, tag="pt", name=f"pt{g}")
                pt_of_job[g] = pt
                if g == 0:
                    # quarter-width matmuls to prime the pipeline
                    for qo in range(0, L, 256):
                        nc.tensor.matmul(sp[:, qo:qo + 256], ktm_sl(b, kt),
                                         qt[:, qo:qo + 256],
                                         start=True, stop=True)
                else:
                    for qo in range(0, L, 512):
                        nc.tensor.matmul(sp[:, qo:qo + 512], ktm_sl(b, kt),
                                         qt[:, qo:qo + 512],
                                         start=True, stop=True)
                if dve_tile[g] and g > 0:
                    nc.vector.tensor_scalar(
                        pt.bitcast(I16)[:], sp[:], EXP_A, EXP_B,
                        mybir.AluOpType.mult, mybir.AluOpType.add)
                elif g == 0:
                    # split exp so the first O-matmuls can start sooner
                    for eo in range(0, L, 512):
                        nc.scalar.activation(
                            pt[:, eo:eo + 512], sp[:, eo:eo + 512],
                            mybir.ActivationFunctionType.Exp)
                else:
                    nc.scalar.activation(pt[:], sp[:],
                                         mybir.ActivationFunctionType.Exp)

            def o_mm(g):
                b, kt = jobs[g]
                nkt = ns[b]
                if kt == 0:
                    # two independent 1-bank accumulator lines so batch b+1's
                    # bank-h accumulate waits only on bank-h's copy of batch b
                    opsum[b] = [
                        op_pool.tile([128, 512], F32, tag=f"oph{h}",
                                     name=f"op{b}h{h}")
                        for h in range(2)
                    ]
                pt = pt_of_job[g]
                last = kt == nkt - 1
                if kt == 0 and (b == 0 or b == BPC - 1):
                    # PSUM is undefined at program start (b==0), and the
                    # last batch opts out of the stale-accumulate chain so
                    # its output is exact fp16: open each bank with a
                    # zero-padded matmul (group 4h's data + zeroed region).
                    # All other batches accumulate onto the previous batch's
                    # (already copied-out) results; the host undoes the
                    # running sum by differencing.
                    vz = vp_s[b][:, :260]  # [V|1|zeros(195)]
                    for h in range(2):
                        nc.tensor.matmul(
                            opsum[b][h][:, :260],
                            pt[:, h * 512:h * 512 + 128], vz,
                            start=True, stop=False,
                            skip_group_check=True)
                    qlist = [1, 2, 3, 5, 6, 7]
                else:
                    qlist = list(range(8))
                vt = vp_s[b][:, kt * (D + 1) + 195:kt * (D + 1) + 260] \
                    if kt > 0 else vp_s[b][:, :D + 1]
                for q in qlist:
                    h, j = divmod(q, 4)
                    nc.tensor.matmul(
                        opsum[b][h][:, j * (D + 1):(j + 1) * (D + 1)],
                        pt[:, q * 128:(q + 1) * 128],
                        vt,
                        start=False,
                        stop=last and (j == 3),
                        skip_group_check=True,
                    )
                pt_of_job[g] = None
                if last and b == BPC - 1:
                    # tail: per-bank copies in parallel (ACT || DVE), fp16
                    osbL = osb_pool.tile([128, 2, 4 * (D + 1)], F16,
                                         tag="osbL", name="osbL")
                    nc.scalar.copy(osbL[:, 0, :], opsum[b][0][:, :4 * (D + 1)])
                    nc.vector.tensor_copy(osbL[:, 1, :],
                                          opsum[b][1][:, :4 * (D + 1)])
                    nc.sync.dma_start(o16_d[:], osbL[:])
                elif last:
                    osb = osb_pool.tile([128, 2, 4 * (D + 1)], F32,
                                        tag="osb", name=f"osb{b}")
                    nc.scalar.copy(osb[:, 0, :], opsum[b][0][:, :4 * (D + 1)])
                    nc.vector.tensor_copy(osb[:, 1, :],
                                          opsum[b][1][:, :4 * (D + 1)])
                    # one DMA per batch, alternating queues (Pool SWDGE / SP)
                    # so neither queue's serial ~1-2us per DMA piles up into
                    # an end-of-kernel drain tail
                    if b % 2 == 0:
                        nc.gpsimd.dma_start(o_d[b], osb[:])
                    else:
                        nc.sync.dma_start(o_d[b], osb[:])

            # pipeline: O of job g trails the S/exp stream by LOOKAHEAD jobs
            # so the PE wait-queue never blocks on an in-flight exp. O (and
            # the trailing copies) are emitted BEFORE the S/exp at the same
            # position so copies land ahead of exp in the ACT/DVE queues.
            LOOKAHEAD = 5
            for g in range(n_jobs):
                if g >= LOOKAHEAD:
                    o_mm(g - LOOKAHEAD)
                s_and_exp(g)
            for g in range(max(0, n_jobs - LOOKAHEAD), n_jobs):
                o_mm(g)

    nc.compile()
    return nc


def get_program(ns):
    ns = tuple(ns)
    if ns not in _prog_cache:
        _prog_cache[ns] = _build_program(ns)
    return _prog_cache[ns]


def _prep_inputs(q, k, v, vl):
    """q,k,v: [n, L, D] fp32; vl: [n] int. Returns (qkm, vp) fp16 arrays."""
    n = q.shape[0]
    qt = np.empty((n, D + 1, L), np.float16)
    qt[:, :D] = (q.transpose(0, 2, 1) * np.float32(0.125)).astype(np.float16)
    qt[:, D] = 1.0
    ktm = np.empty((n, D + 1, L), np.float16)
    ktm[:, :D] = k.transpose(0, 2, 1).astype(np.float16)
    iota = np.arange(L)
    # -1e6 overflows fp16; -60000 is finite in fp16 and still drives both
    # exp paths (ACT underflow, DVE int16 saturation) to exactly 0.
    ktm[:, D] = np.where(iota[None, :] < vl[:, None],
                         np.float16(0.0), np.float16(-60000.0))
    # valid_len == 0: reference softmaxes constant -1e6 -> uniform. Reproduce
    # by zeroing the logits (Q rows and mask row) -> exp(0) everywhere.
    zmask = vl == 0
    if zmask.any():
        qt[zmask, :D] = 0.0
        ktm[zmask, D] = 0.0
    qkm = np.zeros((n, D + 1, QKM_W), np.float16)
    qkm[:, :, :128] = ktm[:, :, :128]
    qkm[:, :, 128:128 + L] = qt
    qkm[:, :, 128 + L:] = ktm[:, :, 128:]
    vpe = np.empty((n, L, D + 1), np.float16)
    vpe[:, :, :D] = v.astype(np.float16)
    vpe[:, :, D] = 1.0
    vpe = vpe.reshape(n, KT, 128, D + 1).transpose(0, 2, 1, 3)
    # pack [vt0 | zeros(195) | vt1 | vt2 | ...] per partition row
    vp = np.zeros((n, 128, 260 + (KT - 1) * (D + 1)), np.float16)
    vp[:, :, :D + 1] = vpe[:, :, 0, :]
    vp[:, :, 260:] = vpe[:, :, 1:, :].reshape(n, 128, (KT - 1) * (D + 1))
    return qkm, vp


def kernel(queries, keys, values, valid_lens):
    queries = np.asarray(queries, np.float32)
    keys = np.asarray(keys, np.float32)
    values = np.asarray(values, np.float32)
    vl = np.asarray(valid_lens).astype(np.int64)

    # Ragged load balancing: sort batches by active k-tile count, deal
    # across cores (slot s <- sorted group) so every core runs the same
    # per-slot tile counts. Slot order (big, small..., big): a big first
    # batch covers the early input-DMA ramp for the small batches queued
    # behind it, and a big last batch leaves only one drain chain exposed.
    nact = np.where(vl == 0, KT, -(-vl // 128)).astype(np.int64)
    order = np.argsort(nact, kind="stable")
    # big batch first (covers the early input-DMA ramp so the saturated DVE
    # engine starts sooner), small batches mid, big batch last (one exposed
    # drain chain)
    slot_groups = [BPC - 2] + list(range(BPC - 2)) + [BPC - 1]
    ns = tuple(int(nact[order[g * N_CORES + N_CORES - 1]])
               for g in slot_groups)

    qkm, vp = _prep_inputs(queries[order], keys[order], values[order],
                           vl[order])

    nc = get_program(ns)
    in_maps = []
    for c in range(N_CORES):
        idx = [slot_groups[s] * N_CORES + c for s in range(BPC)]
        in_maps.append({
            "qkm": np.ascontiguousarray(qkm[idx]),
            "vp": np.ascontiguousarray(vp[idx]),
        })

    res = None
    for attempt in range(3):
        try:
            res = run_bass_kernel_spmd(nc, in_maps, list(range(N_CORES)))
            break
        except Exception:
            # Transient NRT/axon device failures have been observed on the
            # first execution of a freshly compiled NEFF; reset and retry.
            if attempt == 2:
                raise
            import time as _time
            _time.sleep(2.0)
            try:
                import jax
                jax.clear_caches()
            except Exception:
                pass

    out = np.empty((B, L, D), np.float32)
    for c in range(N_CORES):
        raw = res.results[c]["o"][:BPC - 1]  # [BPC-1,128,2,4*65] cumulative
        # slots 0..BPC-2: difference to recover per-slot values; the last
        # slot zero-opened its banks and arrives exact in fp16
        o = np.concatenate([
            raw[:1], np.diff(raw, axis=0),
            res.results[c]["o16"][None].astype(np.float32),
        ])
        o = o.reshape(BPC, 128, 2, 4, D + 1).transpose(0, 2, 3, 1, 4)
        o = o.reshape(BPC, L, D + 1)  # rows [(4h+j)*128 + p]
        on = o[:, :, :D] / o[:, :, D:D + 1]
        for s in range(BPC):
            out[order[slot_groups[s] * N_CORES + c]] = on[s]
    return out
